# revision 1
# baseline (speedup 1.0000x reference)
"""Trainium2 Bass kernel for nn_CoverageLoss.

Math: the reference loss per fragment point is

    min over boxes b of ( min-dist^2 to 100 boundary samples of b ) * outside(b)

The 100 boundary samples are 25 uniformly-spaced points (t = k/24) on each of
the 4 box edges, so the min over samples of one edge has a closed form via
clamped rounding: for the two vertical edges the x-term is (|fx-xc| - w/2)^2
and the y-term is cy^2 with cy = dyl - clamp(round(24*dyl/h), 0, 24) * h/24.
min_b (dist_b * outside_b) == 0 if the point is inside any box, else the plain
min of distances - so the mask becomes "add ~1e30 unless inside both slabs"
folded into the overall min.

Sharding: data-parallel over images; core k handles images [4k, 4k+4) and
their 32 boxes.  Per core the 32768 (point, box) pairs are laid out as
[128 partitions = (box b:8 outer, q=(image n:4, chunk c:4)), 256 points]
fp32 tiles; fragment coords arrive pre-replicated to the 8 b-row groups
(host-side layout) so one unit-stride DMA per coordinate loads them.  The box-min runs after a PE transpose (points onto
partitions) as a strided free-dim reduce, and a ones-matmul collapses the
final partition sum, so each core emits one scalar; the host adds 8 scalars
and scales (the unshard of the mean).
"""

import numpy as np
from contextlib import ExitStack

import concourse.bass as bass
import concourse.bacc as bacc
import concourse.tile as tile
from concourse import masks, mybir
from concourse.bass_utils import run_bass_kernel_spmd

# problem shape (hardcoded per the harness contract)
N_CORES = 8
N_IMG = 32            # total images
NI = N_IMG // N_CORES  # images per core = 4
BPI = 8               # boxes per image
F, FP = 16, 64        # fragments per image, points per fragment
PTS = F * FP          # 1024 points per image
CH = 4                # chunks per image
CW = PTS // CH        # 256 points per chunk
ROWS = NI * CH        # 16 (n, c) rows

DELTA = np.float32(1.0 / 24.0)
MAGIC = float(np.float32(2.0 ** 23))
BIG = float(np.float32(1.0e30))

FP32 = mybir.dt.float32
OP = mybir.AluOpType
AF = mybir.ActivationFunctionType


def build_nc(sim_safe=False):
    """sim_safe is accepted for compatibility; the build is identical."""
    nc = bacc.Bacc("TRN2", target_bir_lowering=False, debug=False)

    frag2 = nc.dram_tensor("frag2", [2, 128, CW], FP32, kind="ExternalInput").ap()
    boxp = nc.dram_tensor("boxp", [128, 4], FP32, kind="ExternalInput").ap()
    out = nc.dram_tensor("out", [1, 1], FP32, kind="ExternalOutput").ap()

    with tile.TileContext(nc) as tc:
        # the race detector collapses strided cross-tensor DMA writes into
        # one shadow and false-positives; Tile still emits all semaphores
        tc.race_detector_enabled = False
        with ExitStack() as ctx:
            pool = ctx.enter_context(tc.tile_pool(name="main", bufs=1))

            def t128(tag, w=CW):
                return pool.tile([128, w], FP32, tag=tag, name=tag)

            # ---- box scalar prep ([128, 2] columns = x, y axis) ----
            bx = pool.tile([128, 4], FP32, tag="bx", name="bx")
            nc.sync.dma_start(bx[:], boxp[:])
            ctr = bx[:, 0:2]
            sz = bx[:, 2:4]
            half = pool.tile([128, 2], FP32, tag="half", name="half")     # w/2, h/2
            nc.vector.tensor_scalar(half[:], sz, 0.5, None, OP.mult)
            lo = pool.tile([128, 2], FP32, tag="lo", name="lo")           # lox, loy
            nc.vector.tensor_tensor(lo[:], ctr, half[:], OP.subtract)
            rec = pool.tile([128, 2], FP32, tag="rec", name="rec")        # 1/w, 1/h
            nc.vector.reciprocal(rec[:], sz)
            winv = pool.tile([128, 2], FP32, tag="winv", name="winv")     # 24/w, 24/h
            nc.vector.tensor_scalar(winv[:], rec[:], 24.0, None, OP.mult)
            nlw = pool.tile([128, 2], FP32, tag="nlw", name="nlw")        # -lo * winv
            nc.vector.scalar_tensor_tensor(nlw[:], lo[:], -1.0, winv[:],
                                           OP.mult, OP.mult)
            nctr = pool.tile([128, 2], FP32, tag="nctr", name="nctr")     # -xc, -yc
            nc.vector.tensor_scalar(nctr[:], ctr, -1.0, None, OP.mult)
            nhalf = pool.tile([128, 2], FP32, tag="nhalf", name="nhalf")  # -w/2
            nc.vector.tensor_scalar(nhalf[:], half[:], -1.0, None, OP.mult)
            nhB = pool.tile([128, 2], FP32, tag="nhB", name="nhB")        # -BIG*w/2
            nc.vector.tensor_scalar(nhB[:], half[:], -BIG, None, OP.mult)
            wd = pool.tile([128, 2], FP32, tag="wd", name="wd")           # w/24
            nc.vector.tensor_scalar(wd[:], sz, float(DELTA), None, OP.mult)

            # ---- fragment coords, broadcast to the 8 box rows per q ----
            fx = t128("fx")
            fy = t128("fy")
            nc.sync.dma_start(fx[:], frag2[0])
            nc.scalar.dma_start(fy[:], frag2[1])

            # ---- per-axis pipelines ----
            def axis_pipe(f, col, sq_tag):
                cs = slice(col, col + 1)
                # s0 = max(0, f*(24/w) - lo*(24/w))    (ACT)
                s0 = t128("s0" + sq_tag)
                nc.scalar.activation(s0[:], f[:], AF.Relu,
                                     bias=nlw[:, cs], scale=winv[:, cs])
                # q1 = round(min(s0, 24)) + MAGIC      (DVE; RNE via magic add)
                q1 = t128("q1" + sq_tag)
                nc.vector.tensor_scalar(q1[:], s0[:], 24.0, MAGIC, OP.min, OP.add)
                # ox = (q1 - MAGIC) * (w/24) = k*w/24  (DVE; q1-MAGIC exact;
                # note: imm scalar1 + AP scalar2 works on HW, two APs do not)
                ox = t128("ox" + sq_tag)
                nc.vector.tensor_scalar(ox[:], q1[:], MAGIC, wd[:, cs],
                                        OP.subtract, OP.mult)
                # tx = ox - f                          (DVE)
                tx = t128("tx" + sq_tag)
                nc.vector.tensor_tensor(tx[:], ox[:], f[:], OP.subtract)
                # qc = (tx + lo)^2 = (f - lo - ox)^2   (ACT)
                qc = t128("qc" + sq_tag)
                nc.scalar.activation(qc[:], tx[:], AF.Square, bias=lo[:, cs])
                # au = |f - ctr|                       (ACT)
                au = t128("au" + sq_tag)
                nc.scalar.activation(au[:], f[:], AF.Abs, bias=nctr[:, cs])
                # e = (au - half)^2 : min over the 2 parallel edges  (ACT)
                e = t128("e" + sq_tag)
                nc.scalar.activation(e[:], au[:], AF.Square, bias=nhalf[:, cs])
                # zx = relu(BIG*au - BIG*half) : >0 iff outside slab (ACT)
                zz = t128("z" + sq_tag)
                nc.scalar.activation(zz[:], au[:], AF.Relu,
                                     bias=nhB[:, cs], scale=BIG)
                return e, qc, zz

            ex, qcx, zx = axis_pipe(fx, 0, "x")
            ey, qcy, zy = axis_pipe(fy, 1, "y")

            # ---- combine ----
            zs = t128("zs")
            nc.vector.tensor_tensor(zs[:], zx[:], zy[:], OP.add)
            e1 = t128("e1")
            nc.vector.tensor_tensor(e1[:], ex[:], qcy[:], OP.add)
            e2 = t128("e2")
            nc.vector.tensor_tensor(e2[:], ey[:], qcx[:], OP.add)
            dmin = t128("dmin")
            nc.vector.tensor_tensor(dmin[:], e1[:], e2[:], OP.min)
            dz = t128("dz")
            nc.vector.tensor_tensor(dz[:], dmin[:], zs[:], OP.min)

            # ---- min over the 8 box rows ----
            # Engines cannot combine different partition ranges (walrus
            # requires equal base partitions), so PE-transpose dz to put
            # points on partitions; the box-min becomes a strided free-dim
            # reduce, and a ones-matmul collapses the final partition sum.
            idn = pool.tile([128, 128], FP32, tag="idn", name="idn")
            masks.make_identity(nc, idn[:])
            ones = pool.tile([128, 1], FP32, tag="ones", name="ones")
            nc.gpsimd.memset(ones[:], 1.0)
            with tc.tile_pool(name="psum", bufs=1, space="PSUM") as psum_pool:
                pA = psum_pool.tile([128, 128], FP32, tag="pA", name="pA")
                pB = psum_pool.tile([128, 128], FP32, tag="pB", name="pB")
                nc.tensor.matmul(pA[:], dz[:, 0:128], idn[:], is_transpose=True)
                nc.tensor.matmul(pB[:], dz[:, 128:256], idn[:], is_transpose=True)
                mA = pool.tile([128, 16], FP32, tag="mA", name="mA")
                mB = pool.tile([128, 16], FP32, tag="mB", name="mB")
                nc.vector.tensor_reduce(
                    mA[:], pA.rearrange("p (b q) -> p q b", b=BPI),
                    axis=mybir.AxisListType.X, op=OP.min)
                nc.vector.tensor_reduce(
                    mB[:], pB.rearrange("p (b q) -> p q b", b=BPI),
                    axis=mybir.AxisListType.X, op=OP.min)
                # total = sum of everything: row-sums via stt accumulator,
                # then a ones-matmul collapses the partition dim.
                tmp16 = pool.tile([128, 16], FP32, tag="tmp16", name="tmp16")
                persum = pool.tile([128, 1], FP32, tag="persum", name="persum")
                nc.vector.scalar_tensor_tensor(tmp16[:], mA[:], 0.0, mB[:],
                                               OP.add, OP.add,
                                               accum_out=persum[:])
                pT = psum_pool.tile([1, 1], FP32, tag="pT", name="pT")
                nc.tensor.matmul(pT[:], persum[:], ones[:])
                fin = pool.tile([1, 1], FP32, tag="fin", name="fin")
                nc.scalar.copy(fin[:], pT[:])
                nc.sync.dma_start(out[:], fin[:])

    nc.compile()
    return nc


def build_nc_raw():
    """Hand-scheduled raw-bacc variant: same dataflow as build_nc but with
    manual semaphores - avoids TileContext's ~9.5us tail (drain + barriers +
    per-sem clears).  Each engine increments its own semaphore on every op
    (vd=DVE, asem=ACT, ts=PE, gs=GPSIMD) and consumers wait on absolute
    producer indices; DMA completions use dedicated sems (+16 per DMA)."""
    nc = bacc.Bacc("TRN2", target_bir_lowering=False, debug=False)

    frag2 = nc.dram_tensor("frag2", [2, 128, CW], FP32, kind="ExternalInput").ap()
    boxp = nc.dram_tensor("boxp", [128, 4], FP32, kind="ExternalInput").ap()
    out = nc.dram_tensor("out", [1, 1], FP32, kind="ExternalOutput").ap()

    with ExitStack() as ctx:
        def sb(name, shape):
            return ctx.enter_context(nc.sbuf_tensor(name, shape, FP32))

        bx = sb("bx", [128, 4])
        half, lo, rec, winv, nlw, nctr, nhalf, nhB, wd = (
            sb(n, [128, 2]) for n in
            ("half", "lo", "rec", "winv", "nlw", "nctr", "nhalf", "nhB", "wd"))
        fx, fy = sb("fx", [128, CW]), sb("fy", [128, CW])
        s0x, q1x, oxx, txx, qcx, aux, ex, zx = (
            sb(n + "x", [128, CW]) for n in
            ("s0", "q1", "ox", "tx", "qc", "au", "e", "z"))
        s0y, q1y, oyy, tyy, qcy, auy, ey, zy = (
            sb(n + "y", [128, CW]) for n in
            ("s0", "q1", "ox", "tx", "qc", "au", "e", "z"))
        zs, e1, e2, dmin, dz = (sb(n, [128, CW]) for n in
                                ("zs", "e1", "e2", "dmin", "dz"))
        idn = sb("idn", [128, 128])
        ones = sb("ones", [128, 1])
        mA, mB, tmp16 = sb("mA", [128, 16]), sb("mB", [128, 16]), sb("tmp16", [128, 16])
        persum = sb("persum", [128, 1])
        fin = sb("fin", [1, 1])
        pA = ctx.enter_context(nc.psum_tensor([128, 128], FP32))
        pB = ctx.enter_context(nc.psum_tensor([128, 128], FP32))
        pT = ctx.enter_context(nc.psum_tensor([1, 1], FP32))

        sbm = ctx.enter_context(nc.semaphore("sbm"))   # boxp dma
        sxm = ctx.enter_context(nc.semaphore("sxm"))   # fx dma
        sym = ctx.enter_context(nc.semaphore("sym"))   # fy dma
        vd = ctx.enter_context(nc.semaphore("vd"))     # DVE op counter
        asem = ctx.enter_context(nc.semaphore("asem")) # ACT op counter
        ts = ctx.enter_context(nc.semaphore("ts"))     # PE op counter
        gs = ctx.enter_context(nc.semaphore("gs"))     # gpsimd op counter
        so = ctx.enter_context(nc.semaphore("so"))     # out dma

        with nc.Block() as block:

            @block.gpsimd
            def _(g):
                g.dma_start(bx[:], boxp[:]).then_inc(sbm, 16)
                g.memset(idn[:], 0.0).then_inc(gs)                      # g1
                g.wait_ge(gs, 1)
                g.affine_select(out=idn[:], in_=idn[:],
                                compare_op=OP.not_equal, fill=1.0, base=0,
                                pattern=[[-1, 128]],
                                channel_multiplier=1).then_inc(gs)      # g2
                g.memset(ones[:], 1.0).then_inc(gs)                     # g3

            @block.sync
            def _(s):
                s.dma_start(fx[:], frag2[0]).then_inc(sxm, 16)
                s.wait_ge(vd, 24)
                s.dma_start(out[:], fin[:]).then_inc(so, 16)
                s.wait_ge(so, 16)

            @block.scalar
            def _(a):
                act = nc.scalar.activation
                a.dma_start(fy[:], frag2[1]).then_inc(sym, 16)
                a.wait_ge(vd, 9)       # all [128,2] preps done
                a.wait_ge(sxm, 16)
                act(s0x[:], fx[:], AF.Relu,
                    bias=nlw[:, 0:1], scale=winv[:, 0:1]).then_inc(asem)    # a1
                act(aux[:], fx[:], AF.Abs, bias=nctr[:, 0:1]).then_inc(asem)  # a2
                a.wait_ge(asem, 2)
                act(ex[:], aux[:], AF.Square, bias=nhalf[:, 0:1]).then_inc(asem)  # a3
                act(zx[:], aux[:], AF.Relu,
                    bias=nhB[:, 0:1], scale=BIG).then_inc(asem)             # a4
                a.wait_ge(sym, 16)
                act(s0y[:], fy[:], AF.Relu,
                    bias=nlw[:, 1:2], scale=winv[:, 1:2]).then_inc(asem)    # a5
                act(auy[:], fy[:], AF.Abs, bias=nctr[:, 1:2]).then_inc(asem)  # a6
                a.wait_ge(asem, 6)
                act(ey[:], auy[:], AF.Square, bias=nhalf[:, 1:2]).then_inc(asem)  # a7
                act(zy[:], auy[:], AF.Relu,
                    bias=nhB[:, 1:2], scale=BIG).then_inc(asem)             # a8
                a.wait_ge(vd, 12)
                act(qcx[:], txx[:], AF.Square, bias=lo[:, 0:1]).then_inc(asem)   # a9
                a.wait_ge(vd, 15)
                act(qcy[:], tyy[:], AF.Square, bias=lo[:, 1:2]).then_inc(asem)   # a10

            @block.vector
            def _(v):
                V = nc.vector
                v.wait_ge(sbm, 16)
                ctr, sz = bx[:, 0:2], bx[:, 2:4]
                V.tensor_scalar(half[:], sz, 0.5, None, OP.mult).then_inc(vd)      # 1
                v.wait_ge(vd, 1)
                V.tensor_tensor(lo[:], ctr, half[:], OP.subtract).then_inc(vd)     # 2
                V.reciprocal(rec[:], sz).then_inc(vd)                              # 3
                v.wait_ge(vd, 3)
                V.tensor_scalar(winv[:], rec[:], 24.0, None, OP.mult).then_inc(vd)  # 4
                v.wait_ge(vd, 4)
                V.scalar_tensor_tensor(nlw[:], lo[:], -1.0, winv[:],
                                       OP.mult, OP.mult).then_inc(vd)              # 5
                V.tensor_scalar(nctr[:], ctr, -1.0, None, OP.mult).then_inc(vd)    # 6
                V.tensor_scalar(nhalf[:], half[:], -1.0, None, OP.mult).then_inc(vd)  # 7
                V.tensor_scalar(nhB[:], half[:], -BIG, None, OP.mult).then_inc(vd)    # 8
                V.tensor_scalar(wd[:], sz, float(DELTA), None, OP.mult).then_inc(vd)  # 9
                v.wait_ge(asem, 1)
                V.tensor_scalar(q1x[:], s0x[:], 24.0, MAGIC, OP.min,
                                OP.add).then_inc(vd)                               # 10
                v.wait_ge(vd, 10)
                V.tensor_scalar(oxx[:], q1x[:], MAGIC, wd[:, 0:1],
                                OP.subtract, OP.mult).then_inc(vd)                 # 11
                v.wait_ge(vd, 11)
                v.wait_ge(sxm, 16)
                V.tensor_tensor(txx[:], oxx[:], fx[:], OP.subtract).then_inc(vd)   # 12
                v.wait_ge(asem, 5)
                V.tensor_scalar(q1y[:], s0y[:], 24.0, MAGIC, OP.min,
                                OP.add).then_inc(vd)                               # 13
                v.wait_ge(vd, 13)
                V.tensor_scalar(oyy[:], q1y[:], MAGIC, wd[:, 1:2],
                                OP.subtract, OP.mult).then_inc(vd)                 # 14
                v.wait_ge(vd, 14)
                v.wait_ge(sym, 16)
                V.tensor_tensor(tyy[:], oyy[:], fy[:], OP.subtract).then_inc(vd)   # 15
                v.wait_ge(asem, 8)
                V.tensor_tensor(zs[:], zx[:], zy[:], OP.add).then_inc(vd)          # 16
                v.wait_ge(asem, 10)
                V.tensor_tensor(e1[:], ex[:], qcy[:], OP.add).then_inc(vd)         # 17
                V.tensor_tensor(e2[:], ey[:], qcx[:], OP.add).then_inc(vd)         # 18
                v.wait_ge(vd, 18)
                V.tensor_tensor(dmin[:], e1[:], e2[:], OP.min).then_inc(vd)        # 19
                v.wait_ge(vd, 19)
                V.tensor_tensor(dz[:], dmin[:], zs[:], OP.min).then_inc(vd)        # 20
                v.wait_ge(ts, 1)
                V.tensor_reduce(
                    mA[:], pA[:].rearrange("p (b q) -> p q b", b=BPI),
                    axis=mybir.AxisListType.X, op=OP.min).then_inc(vd)             # 21
                v.wait_ge(ts, 2)
                V.tensor_reduce(
                    mB[:], pB[:].rearrange("p (b q) -> p q b", b=BPI),
                    axis=mybir.AxisListType.X, op=OP.min).then_inc(vd)             # 22
                v.wait_ge(vd, 22)
                V.scalar_tensor_tensor(tmp16[:], mA[:], 0.0, mB[:],
                                       OP.add, OP.add,
                                       accum_out=persum[:]).then_inc(vd)           # 23
                v.wait_ge(ts, 3)
                V.tensor_copy(fin[:], pT[:]).then_inc(vd)                          # 24

            @block.tensor
            def _(t):
                t.wait_ge(gs, 2)
                t.wait_ge(vd, 20)
                nc.tensor.matmul(pA[:], dz[:, 0:128], idn[:],
                                 is_transpose=True).then_inc(ts)
                nc.tensor.matmul(pB[:], dz[:, 128:256], idn[:],
                                 is_transpose=True).then_inc(ts)
                t.wait_ge(gs, 3)
                t.wait_ge(vd, 23)
                nc.tensor.matmul(pT[:], persum[:], ones[:]).then_inc(ts)

    nc.compile()
    return nc


# partition row p = b*16 + q, q = n*4 + c
_P = np.arange(128)
_B_IDX = _P // (NI * CH)
_N_IDX = (_P % (NI * CH)) // CH


def shard_inputs(boxes, fragments):
    """Per-core input marshalling (layout only, no arithmetic)."""
    boxes = np.ascontiguousarray(boxes, dtype=np.float32).reshape(
        N_CORES, NI, BPI, 4)
    frag = np.ascontiguousarray(fragments, dtype=np.float32).reshape(
        N_CORES, NI, CH, CW, 2)
    in_maps = []
    for k in range(N_CORES):
        f2 = frag[k].transpose(3, 0, 1, 2).reshape(2, ROWS, CW)
        frag2 = np.ascontiguousarray(
            np.broadcast_to(f2[:, None], (2, BPI, ROWS, CW)).reshape(2, 128, CW))
        boxp = np.ascontiguousarray(boxes[k, _N_IDX, _B_IDX, :])
        in_maps.append({"frag2": frag2, "boxp": boxp})
    return in_maps


_NC = None


import os


def _get_nc():
    global _NC
    if _NC is None:
        if os.environ.get("COV_RAW_KERNEL"):
            _NC = build_nc_raw()
        else:
            _NC = build_nc()
    return _NC


def run(boxes, fragments, trace=False, **spmd_kwargs):
    nc = _get_nc()
    in_maps = shard_inputs(boxes, fragments)
    res = run_bass_kernel_spmd(nc, in_maps, list(range(N_CORES)),
                               trace=trace, **spmd_kwargs)
    parts = np.stack([np.asarray(r["out"], dtype=np.float32).reshape(-1)
                      for r in res.results])
    total = parts.sum(dtype=np.float32)
    loss = np.float32(total / np.float32(FP * N_IMG))
    return loss, res


def kernel(boxes, fragments, obj_to_img):
    loss, _ = run(boxes, fragments)
    return loss



# revision 5
# speedup vs baseline: 1.1273x; 1.1273x over previous
"""Trainium2 Bass kernel for nn_CoverageLoss.

Math: the reference loss per fragment point is

    min over boxes b of ( min-dist^2 to 100 boundary samples of b ) * outside(b)

The 100 boundary samples are 25 uniformly-spaced points (t = k/24) on each of
the 4 box edges, so the min over samples of one edge has a closed form via
clamped rounding: for the two vertical edges the x-term is (|fx-xc| - w/2)^2
and the y-term is cy^2 with cy = dyl - clamp(round(24*dyl/h), 0, 24) * h/24.
min_b (dist_b * outside_b) == 0 if the point is inside any box, else the plain
min of distances - so the mask becomes "add ~1e30 unless inside both slabs"
folded into the overall min.

Sharding: data-parallel over images; core k handles images [4k, 4k+4) and
their 32 boxes.  Per core the 32768 (point, box) pairs are laid out as
[128 partitions = (box b:8 outer, q=(image n:4, chunk c:4)), 256 points]
fp32 tiles; fragment coords arrive pre-replicated to the 8 b-row groups
(host-side layout) so one unit-stride DMA per coordinate loads them.  The box-min runs after a PE transpose (points onto
partitions) as a strided free-dim reduce, and a ones-matmul collapses the
final partition sum, so each core emits one scalar; the host adds 8 scalars
and scales (the unshard of the mean).
"""

import numpy as np
from contextlib import ExitStack

import concourse.bass as bass
import concourse.bacc as bacc
import concourse.tile as tile
from concourse import masks, mybir
from concourse.bass_utils import run_bass_kernel_spmd

# problem shape (hardcoded per the harness contract)
N_CORES = 8
N_IMG = 32            # total images
NI = N_IMG // N_CORES  # images per core = 4
BPI = 8               # boxes per image
F, FP = 16, 64        # fragments per image, points per fragment
PTS = F * FP          # 1024 points per image
CH = 4                # chunks per image
CW = PTS // CH        # 256 points per chunk
ROWS = NI * CH        # 16 (n, c) rows

DELTA = np.float32(1.0 / 24.0)
MAGIC = float(np.float32(2.0 ** 23))
BIG = float(np.float32(1.0e30))

FP32 = mybir.dt.float32
OP = mybir.AluOpType
AF = mybir.ActivationFunctionType


def build_nc(sim_safe=False):
    """sim_safe is accepted for compatibility; the build is identical."""
    nc = bacc.Bacc("TRN2", target_bir_lowering=False, debug=False)

    frag2 = nc.dram_tensor("frag2", [2, 128, CW], FP32, kind="ExternalInput").ap()
    boxp = nc.dram_tensor("boxp", [128, 4], FP32, kind="ExternalInput").ap()
    out = nc.dram_tensor("out", [1, 1], FP32, kind="ExternalOutput").ap()

    with tile.TileContext(nc) as tc:
        # the race detector collapses strided cross-tensor DMA writes into
        # one shadow and false-positives; Tile still emits all semaphores
        tc.race_detector_enabled = False
        with ExitStack() as ctx:
            pool = ctx.enter_context(tc.tile_pool(name="main", bufs=1))

            def t128(tag, w=CW):
                return pool.tile([128, w], FP32, tag=tag, name=tag)

            # ---- box scalar prep ([128, 2] columns = x, y axis) ----
            bx = pool.tile([128, 4], FP32, tag="bx", name="bx")
            nc.sync.dma_start(bx[:], boxp[:])
            ctr = bx[:, 0:2]
            sz = bx[:, 2:4]
            half = pool.tile([128, 2], FP32, tag="half", name="half")     # w/2, h/2
            nc.vector.tensor_scalar(half[:], sz, 0.5, None, OP.mult)
            lo = pool.tile([128, 2], FP32, tag="lo", name="lo")           # lox, loy
            nc.vector.tensor_tensor(lo[:], ctr, half[:], OP.subtract)
            rec = pool.tile([128, 2], FP32, tag="rec", name="rec")        # 1/w, 1/h
            nc.vector.reciprocal(rec[:], sz)
            winv = pool.tile([128, 2], FP32, tag="winv", name="winv")     # 24/w, 24/h
            nc.vector.tensor_scalar(winv[:], rec[:], 24.0, None, OP.mult)
            nlw = pool.tile([128, 2], FP32, tag="nlw", name="nlw")        # -lo * winv
            nc.vector.scalar_tensor_tensor(nlw[:], lo[:], -1.0, winv[:],
                                           OP.mult, OP.mult)
            nctr = pool.tile([128, 2], FP32, tag="nctr", name="nctr")     # -xc, -yc
            nc.vector.tensor_scalar(nctr[:], ctr, -1.0, None, OP.mult)
            nhalf = pool.tile([128, 2], FP32, tag="nhalf", name="nhalf")  # -w/2
            nc.vector.tensor_scalar(nhalf[:], half[:], -1.0, None, OP.mult)
            nhB = pool.tile([128, 2], FP32, tag="nhB", name="nhB")        # -BIG*w/2
            nc.vector.tensor_scalar(nhB[:], half[:], -BIG, None, OP.mult)
            wd = pool.tile([128, 2], FP32, tag="wd", name="wd")           # w/24
            nc.vector.tensor_scalar(wd[:], sz, float(DELTA), None, OP.mult)

            # ---- fragment coords, broadcast to the 8 box rows per q ----
            fx = t128("fx")
            fy = t128("fy")
            nc.sync.dma_start(fx[:], frag2[0])
            nc.scalar.dma_start(fy[:], frag2[1])

            # ---- per-axis pipelines ----
            def axis_pipe(f, col, sq_tag):
                cs = slice(col, col + 1)
                # s0 = max(0, f*(24/w) - lo*(24/w))    (ACT)
                s0 = t128("s0" + sq_tag)
                nc.scalar.activation(s0[:], f[:], AF.Relu,
                                     bias=nlw[:, cs], scale=winv[:, cs])
                # q1 = round(min(s0, 24)) + MAGIC      (DVE; RNE via magic add)
                q1 = t128("q1" + sq_tag)
                nc.vector.tensor_scalar(q1[:], s0[:], 24.0, MAGIC, OP.min, OP.add)
                # ox = (q1 - MAGIC) * (w/24) = k*w/24  (DVE; q1-MAGIC exact;
                # note: imm scalar1 + AP scalar2 works on HW, two APs do not)
                ox = t128("ox" + sq_tag)
                nc.vector.tensor_scalar(ox[:], q1[:], MAGIC, wd[:, cs],
                                        OP.subtract, OP.mult)
                # tx = ox - f                          (DVE)
                tx = t128("tx" + sq_tag)
                nc.vector.tensor_tensor(tx[:], ox[:], f[:], OP.subtract)
                # qc = (tx + lo)^2 = (f - lo - ox)^2   (ACT)
                qc = t128("qc" + sq_tag)
                nc.scalar.activation(qc[:], tx[:], AF.Square, bias=lo[:, cs])
                # au = |f - ctr|                       (ACT)
                au = t128("au" + sq_tag)
                nc.scalar.activation(au[:], f[:], AF.Abs, bias=nctr[:, cs])
                # e = (au - half)^2 : min over the 2 parallel edges  (ACT)
                e = t128("e" + sq_tag)
                nc.scalar.activation(e[:], au[:], AF.Square, bias=nhalf[:, cs])
                # zx = relu(BIG*au - BIG*half) : >0 iff outside slab (ACT)
                zz = t128("z" + sq_tag)
                nc.scalar.activation(zz[:], au[:], AF.Relu,
                                     bias=nhB[:, cs], scale=BIG)
                return e, qc, zz

            ex, qcx, zx = axis_pipe(fx, 0, "x")
            ey, qcy, zy = axis_pipe(fy, 1, "y")

            # ---- combine ----
            zs = t128("zs")
            nc.vector.tensor_tensor(zs[:], zx[:], zy[:], OP.add)
            e1 = t128("e1")
            nc.vector.tensor_tensor(e1[:], ex[:], qcy[:], OP.add)
            e2 = t128("e2")
            nc.vector.tensor_tensor(e2[:], ey[:], qcx[:], OP.add)
            dmin = t128("dmin")
            nc.vector.tensor_tensor(dmin[:], e1[:], e2[:], OP.min)
            dz = t128("dz")
            nc.vector.tensor_tensor(dz[:], dmin[:], zs[:], OP.min)

            # ---- min over the 8 box rows ----
            # Engines cannot combine different partition ranges (walrus
            # requires equal base partitions), so PE-transpose dz to put
            # points on partitions; the box-min becomes a strided free-dim
            # reduce, and a ones-matmul collapses the final partition sum.
            idn = pool.tile([128, 128], FP32, tag="idn", name="idn")
            masks.make_identity(nc, idn[:])
            ones = pool.tile([128, 1], FP32, tag="ones", name="ones")
            nc.gpsimd.memset(ones[:], 1.0)
            with tc.tile_pool(name="psum", bufs=1, space="PSUM") as psum_pool:
                pA = psum_pool.tile([128, 128], FP32, tag="pA", name="pA")
                pB = psum_pool.tile([128, 128], FP32, tag="pB", name="pB")
                nc.tensor.matmul(pA[:], dz[:, 0:128], idn[:], is_transpose=True)
                nc.tensor.matmul(pB[:], dz[:, 128:256], idn[:], is_transpose=True)
                mA = pool.tile([128, 16], FP32, tag="mA", name="mA")
                mB = pool.tile([128, 16], FP32, tag="mB", name="mB")
                nc.vector.tensor_reduce(
                    mA[:], pA.rearrange("p (b q) -> p q b", b=BPI),
                    axis=mybir.AxisListType.X, op=OP.min)
                nc.vector.tensor_reduce(
                    mB[:], pB.rearrange("p (b q) -> p q b", b=BPI),
                    axis=mybir.AxisListType.X, op=OP.min)
                # total = sum of everything: row-sums via stt accumulator,
                # then a ones-matmul collapses the partition dim.
                tmp16 = pool.tile([128, 16], FP32, tag="tmp16", name="tmp16")
                persum = pool.tile([128, 1], FP32, tag="persum", name="persum")
                nc.vector.scalar_tensor_tensor(tmp16[:], mA[:], 0.0, mB[:],
                                               OP.add, OP.add,
                                               accum_out=persum[:])
                pT = psum_pool.tile([1, 1], FP32, tag="pT", name="pT")
                nc.tensor.matmul(pT[:], persum[:], ones[:])
                fin = pool.tile([1, 1], FP32, tag="fin", name="fin")
                nc.scalar.copy(fin[:], pT[:])
                nc.sync.dma_start(out[:], fin[:])

    nc.compile()
    return nc


def build_nc_raw():
    """Hand-scheduled raw-bacc variant: same dataflow as build_nc but with
    manual semaphores - avoids TileContext's ~9.5us tail (drain + barriers +
    per-sem clears).  Each engine increments its own semaphore on every op
    (vd=DVE, asem=ACT, ts=PE, gs=GPSIMD) and consumers wait on absolute
    producer indices; DMA completions use dedicated sems (+16 per DMA)."""
    nc = bacc.Bacc("TRN2", target_bir_lowering=False, debug=False)

    frag2 = nc.dram_tensor("frag2", [2, 128, CW], FP32, kind="ExternalInput").ap()
    boxp = nc.dram_tensor("boxp", [128, 4], FP32, kind="ExternalInput").ap()
    out = nc.dram_tensor("out", [1, 1], FP32, kind="ExternalOutput").ap()

    with ExitStack() as ctx:
        def sb(name, shape):
            return ctx.enter_context(nc.sbuf_tensor(name, shape, FP32))

        bx = sb("bx", [128, 4])
        half, lo, rec, winv, nlw, nctr, nhalf, nhB, wd = (
            sb(n, [128, 2]) for n in
            ("half", "lo", "rec", "winv", "nlw", "nctr", "nhalf", "nhB", "wd"))
        fx, fy = sb("fx", [128, CW]), sb("fy", [128, CW])
        s0x, q1x, oxx, txx, qcx, aux, ex, zx = (
            sb(n + "x", [128, CW]) for n in
            ("s0", "q1", "ox", "tx", "qc", "au", "e", "z"))
        s0y, q1y, oyy, tyy, qcy, auy, ey, zy = (
            sb(n + "y", [128, CW]) for n in
            ("s0", "q1", "ox", "tx", "qc", "au", "e", "z"))
        zs, e1, e2, dmin, dz = (sb(n, [128, CW]) for n in
                                ("zs", "e1", "e2", "dmin", "dz"))
        idn = sb("idn", [128, 128])
        ones = sb("ones", [128, 1])
        mA, mB, tmp16 = sb("mA", [128, 16]), sb("mB", [128, 16]), sb("tmp16", [128, 16])
        persum = sb("persum", [128, 1])
        fin = sb("fin", [1, 1])
        pA = ctx.enter_context(nc.psum_tensor([128, 128], FP32))
        pB = ctx.enter_context(nc.psum_tensor([128, 128], FP32))
        pT = ctx.enter_context(nc.psum_tensor([1, 1], FP32))

        sbm = ctx.enter_context(nc.semaphore("sbm"))   # boxp dma
        sxm = ctx.enter_context(nc.semaphore("sxm"))   # fx dma
        sym = ctx.enter_context(nc.semaphore("sym"))   # fy dma
        vd = ctx.enter_context(nc.semaphore("vd"))     # DVE op counter
        asem = ctx.enter_context(nc.semaphore("asem")) # ACT op counter
        ts = ctx.enter_context(nc.semaphore("ts"))     # PE op counter
        gs = ctx.enter_context(nc.semaphore("gs"))     # gpsimd op counter
        so = ctx.enter_context(nc.semaphore("so"))     # out dma

        with nc.Block() as block:

            @block.gpsimd
            def _(g):
                g.dma_start(bx[:], boxp[:]).then_inc(sbm, 16)
                g.memset(idn[:], 0.0).then_inc(gs)                      # g1
                g.wait_ge(gs, 1)
                g.affine_select(out=idn[:], in_=idn[:],
                                compare_op=OP.not_equal, fill=1.0, base=0,
                                pattern=[[-1, 128]],
                                channel_multiplier=1).then_inc(gs)      # g2
                g.memset(ones[:], 1.0).then_inc(gs)                     # g3

            @block.sync
            def _(s):
                s.dma_start(fx[:], frag2[0]).then_inc(sxm, 16)
                s.wait_ge(vd, 24)
                s.dma_start(out[:], fin[:]).then_inc(so, 16)
                s.wait_ge(so, 16)

            @block.scalar
            def _(a):
                act = nc.scalar.activation
                a.dma_start(fy[:], frag2[1]).then_inc(sym, 16)
                a.wait_ge(vd, 9)       # all [128,2] preps done
                a.wait_ge(sxm, 16)
                act(s0x[:], fx[:], AF.Relu,
                    bias=nlw[:, 0:1], scale=winv[:, 0:1]).then_inc(asem)    # a1
                act(aux[:], fx[:], AF.Abs, bias=nctr[:, 0:1]).then_inc(asem)  # a2
                a.wait_ge(asem, 2)
                act(ex[:], aux[:], AF.Square, bias=nhalf[:, 0:1]).then_inc(asem)  # a3
                act(zx[:], aux[:], AF.Relu,
                    bias=nhB[:, 0:1], scale=BIG).then_inc(asem)             # a4
                a.wait_ge(sym, 16)
                act(s0y[:], fy[:], AF.Relu,
                    bias=nlw[:, 1:2], scale=winv[:, 1:2]).then_inc(asem)    # a5
                act(auy[:], fy[:], AF.Abs, bias=nctr[:, 1:2]).then_inc(asem)  # a6
                a.wait_ge(asem, 6)
                act(ey[:], auy[:], AF.Square, bias=nhalf[:, 1:2]).then_inc(asem)  # a7
                act(zy[:], auy[:], AF.Relu,
                    bias=nhB[:, 1:2], scale=BIG).then_inc(asem)             # a8
                a.wait_ge(vd, 12)
                act(qcx[:], txx[:], AF.Square, bias=lo[:, 0:1]).then_inc(asem)   # a9
                a.wait_ge(vd, 15)
                act(qcy[:], tyy[:], AF.Square, bias=lo[:, 1:2]).then_inc(asem)   # a10

            @block.vector
            def _(v):
                V = nc.vector
                v.wait_ge(sbm, 16)
                ctr, sz = bx[:, 0:2], bx[:, 2:4]
                V.tensor_scalar(half[:], sz, 0.5, None, OP.mult).then_inc(vd)      # 1
                v.wait_ge(vd, 1)
                V.tensor_tensor(lo[:], ctr, half[:], OP.subtract).then_inc(vd)     # 2
                V.reciprocal(rec[:], sz).then_inc(vd)                              # 3
                v.wait_ge(vd, 3)
                V.tensor_scalar(winv[:], rec[:], 24.0, None, OP.mult).then_inc(vd)  # 4
                v.wait_ge(vd, 4)
                V.scalar_tensor_tensor(nlw[:], lo[:], -1.0, winv[:],
                                       OP.mult, OP.mult).then_inc(vd)              # 5
                V.tensor_scalar(nctr[:], ctr, -1.0, None, OP.mult).then_inc(vd)    # 6
                V.tensor_scalar(nhalf[:], half[:], -1.0, None, OP.mult).then_inc(vd)  # 7
                V.tensor_scalar(nhB[:], half[:], -BIG, None, OP.mult).then_inc(vd)    # 8
                V.tensor_scalar(wd[:], sz, float(DELTA), None, OP.mult).then_inc(vd)  # 9
                v.wait_ge(asem, 1)
                V.tensor_scalar(q1x[:], s0x[:], 24.0, MAGIC, OP.min,
                                OP.add).then_inc(vd)                               # 10
                v.wait_ge(vd, 10)
                V.tensor_scalar(oxx[:], q1x[:], MAGIC, wd[:, 0:1],
                                OP.subtract, OP.mult).then_inc(vd)                 # 11
                v.wait_ge(vd, 11)
                v.wait_ge(sxm, 16)
                V.tensor_tensor(txx[:], oxx[:], fx[:], OP.subtract).then_inc(vd)   # 12
                v.wait_ge(asem, 5)
                V.tensor_scalar(q1y[:], s0y[:], 24.0, MAGIC, OP.min,
                                OP.add).then_inc(vd)                               # 13
                v.wait_ge(vd, 13)
                V.tensor_scalar(oyy[:], q1y[:], MAGIC, wd[:, 1:2],
                                OP.subtract, OP.mult).then_inc(vd)                 # 14
                v.wait_ge(vd, 14)
                v.wait_ge(sym, 16)
                V.tensor_tensor(tyy[:], oyy[:], fy[:], OP.subtract).then_inc(vd)   # 15
                v.wait_ge(asem, 8)
                V.tensor_tensor(zs[:], zx[:], zy[:], OP.add).then_inc(vd)          # 16
                v.wait_ge(asem, 10)
                V.tensor_tensor(e1[:], ex[:], qcy[:], OP.add).then_inc(vd)         # 17
                V.tensor_tensor(e2[:], ey[:], qcx[:], OP.add).then_inc(vd)         # 18
                v.wait_ge(vd, 18)
                V.tensor_tensor(dmin[:], e1[:], e2[:], OP.min).then_inc(vd)        # 19
                v.wait_ge(vd, 19)
                V.tensor_tensor(dz[:], dmin[:], zs[:], OP.min).then_inc(vd)        # 20
                v.wait_ge(ts, 1)
                V.tensor_reduce(
                    mA[:], pA[:].rearrange("p (b q) -> p q b", b=BPI),
                    axis=mybir.AxisListType.X, op=OP.min).then_inc(vd)             # 21
                v.wait_ge(ts, 2)
                V.tensor_reduce(
                    mB[:], pB[:].rearrange("p (b q) -> p q b", b=BPI),
                    axis=mybir.AxisListType.X, op=OP.min).then_inc(vd)             # 22
                v.wait_ge(vd, 22)
                V.scalar_tensor_tensor(tmp16[:], mA[:], 0.0, mB[:],
                                       OP.add, OP.add,
                                       accum_out=persum[:]).then_inc(vd)           # 23
                v.wait_ge(ts, 3)
                V.tensor_copy(fin[:], pT[:]).then_inc(vd)                          # 24

            @block.tensor
            def _(t):
                t.wait_ge(gs, 2)
                t.wait_ge(vd, 20)
                nc.tensor.matmul(pA[:], dz[:, 0:128], idn[:],
                                 is_transpose=True).then_inc(ts)
                nc.tensor.matmul(pB[:], dz[:, 128:256], idn[:],
                                 is_transpose=True).then_inc(ts)
                t.wait_ge(gs, 3)
                t.wait_ge(vd, 23)
                nc.tensor.matmul(pT[:], persum[:], ones[:]).then_inc(ts)

    nc.compile()
    return nc


def build_nc_v2():
    """fp16 rework of build_nc_raw.

    Math per (box, point): asc = 24*|f - c|/w (grid units).  Then
      qc = ((round(min(asc,12)) - asc) * w/24)^2   nearest-sample distance
      e  = ((asc - 12) * w/24)^2                   slab-edge distance
      z  = (asc - 12) * BIG*w/24                   >0 iff outside the slab
      dz = min(max(max(zx,zy), 0), min(ex+qcy, ey+qcx))
    round() is the fp16 magic-number trick (+1024, RNE); everything after
    the two |.| activations runs in fp16 (2x DVE rate; sim rel err 8e-4
    vs the 2e-2 gate, and asc <= 24/min(w) ~ 2e4 stays under fp16 max).
    Scheduling: ACT gets 6 big ops, DVE 11 + preps, Pool preps the
    identity, PE transposes for the box-min, scalar issues the out DMA.
    """
    nc = bacc.Bacc("TRN2", target_bir_lowering=False, debug=False)

    frag2 = nc.dram_tensor("frag2", [2, 128, CW], FP32, kind="ExternalInput").ap()
    boxp = nc.dram_tensor("boxp", [128, 4], FP32, kind="ExternalInput").ap()
    out = nc.dram_tensor("out", [1, 1], FP32, kind="ExternalOutput").ap()

    FP16 = mybir.dt.float16
    BIGZ = 1.0e4

    with ExitStack() as ctx:
        def sb(name, shape, dt=FP32):
            return ctx.enter_context(nc.sbuf_tensor(name, shape, dt))

        bx = sb("bx", [128, 4])
        rec32 = sb("rec32", [128, 2])
        # AP scalars (DVE scalar2, ACT scale/bias) must be fp32
        winv16, nwc16, nhalf16, wd16, bwd16 = (
            sb(n, [128, 2])
            for n in ("winv16", "nwc16", "nhalf16", "wd16", "bwd16"))
        fxt, fyt = sb("fxt", [128, CW]), sb("fyt", [128, CW])
        ascx, ascy, q1x, q1y, mdx, mdy, qcx, qcy, ex, ey, zx, zy = (
            sb(n, [128, CW], FP16)
            for n in ("ascx", "ascy", "q1x", "q1y", "mdx", "mdy",
                      "qcx", "qcy", "ex", "ey", "zx", "zy"))
        s2, e1, e2, dmin, dz = (sb(n, [128, CW], FP16)
                                for n in ("s2", "e1", "e2", "dmin", "dz"))
        idn16 = sb("idn16", [128, 128], FP16)
        ones = sb("ones", [128, 1])
        mA, mB, tmpm = sb("mA", [128, 16]), sb("mB", [128, 16]), sb("tmpm", [128, 16])
        persum = sb("persum", [128, 1])
        fin = sb("fin", [1, 1])
        pA = ctx.enter_context(nc.psum_tensor([128, 128], FP16))
        pB = ctx.enter_context(nc.psum_tensor([128, 128], FP16))
        pT = ctx.enter_context(nc.psum_tensor([1, 1], FP32))

        sbx = ctx.enter_context(nc.semaphore("sbx"))
        sfx = ctx.enter_context(nc.semaphore("sfx"))
        sfy = ctx.enter_context(nc.semaphore("sfy"))
        so = ctx.enter_context(nc.semaphore("so"))
        vd = ctx.enter_context(nc.semaphore("vd"))
        ad = ctx.enter_context(nc.semaphore("ad"))
        pd = ctx.enter_context(nc.semaphore("pd"))
        ts = ctx.enter_context(nc.semaphore("ts"))

        with nc.Block() as block:

            @block.sync
            def _(s):
                s.dma_start(bx[:], boxp[:]).then_inc(sbx, 16)
                s.dma_start(fyt[:], frag2[1]).then_inc(sfy, 16)

            @block.gpsimd
            def _(g):
                g.memset(idn16[:], 0.0).then_inc(pd)                    # p1
                g.wait_ge(pd, 1)
                g.affine_select(out=idn16[:], in_=idn16[:],
                                compare_op=OP.not_equal, fill=1.0, base=0,
                                pattern=[[-1, 128]],
                                channel_multiplier=1).then_inc(pd)      # p2
                g.memset(ones[:], 1.0).then_inc(pd)                     # p3

            @block.scalar
            def _(a):
                act = nc.scalar.activation
                a.dma_start(fxt[:], frag2[0]).then_inc(sfx, 16)
                a.wait_ge(vd, 3)
                a.wait_ge(sfx, 16)
                act(ascx[:], fxt[:], AF.Abs,
                    bias=nwc16[:, 0:1], scale=winv16[:, 0:1]).then_inc(ad)   # a1
                a.wait_ge(sfy, 16)
                act(ascy[:], fyt[:], AF.Abs,
                    bias=nwc16[:, 1:2], scale=winv16[:, 1:2]).then_inc(ad)   # a2
                a.wait_ge(vd, 8)
                act(qcx[:], mdx[:], AF.Square,
                    scale=wd16[:, 0:1]).then_inc(ad)                         # a3
                act(ex[:], ascx[:], AF.Square,
                    bias=nhalf16[:, 0:1], scale=wd16[:, 0:1]).then_inc(ad)   # a4
                a.wait_ge(vd, 10)
                act(qcy[:], mdy[:], AF.Square,
                    scale=wd16[:, 1:2]).then_inc(ad)                         # a5
                act(ey[:], ascy[:], AF.Square,
                    bias=nhalf16[:, 1:2], scale=wd16[:, 1:2]).then_inc(ad)   # a6
                a.wait_ge(vd, 21)
                a.dma_start(out[:], fin[:]).then_inc(so, 16)
                a.wait_ge(so, 16)

            @block.vector
            def _(v):
                V = nc.vector
                v.wait_ge(sbx, 16)
                ctr, sz = bx[:, 0:2], bx[:, 2:4]
                V.reciprocal(rec32[:], sz).then_inc(vd)                       # 1
                v.wait_ge(vd, 1)
                V.tensor_scalar(winv16[:], rec32[:], 24.0, None,
                                OP.mult).then_inc(vd)                         # 2
                V.scalar_tensor_tensor(nwc16[:], ctr, -24.0, rec32[:],
                                       OP.mult, OP.mult).then_inc(vd)         # 3
                V.tensor_scalar(nhalf16[:], sz, -0.5, None,
                                OP.mult).then_inc(vd)                         # 4
                V.tensor_scalar(wd16[:], sz, float(DELTA), None,
                                OP.mult).then_inc(vd)                         # 5
                V.tensor_scalar(bwd16[:], sz, BIGZ * float(DELTA), None,
                                OP.mult).then_inc(vd)                         # 6
                v.wait_ge(ad, 1)
                V.tensor_scalar(q1x[:], ascx[:], 12.0, 1024.0,
                                OP.min, OP.add).then_inc(vd)                  # 7
                v.wait_ge(vd, 7)
                V.scalar_tensor_tensor(mdx[:], q1x[:], 1024.0, ascx[:],
                                       OP.subtract, OP.subtract).then_inc(vd)  # 8
                v.wait_ge(ad, 2)
                V.tensor_scalar(q1y[:], ascy[:], 12.0, 1024.0,
                                OP.min, OP.add).then_inc(vd)                  # 9
                v.wait_ge(vd, 9)
                V.scalar_tensor_tensor(mdy[:], q1y[:], 1024.0, ascy[:],
                                       OP.subtract, OP.subtract).then_inc(vd)  # 10
                V.tensor_scalar(zx[:], ascx[:], 12.0, bwd16[:, 0:1],
                                OP.subtract, OP.mult).then_inc(vd)            # 11
                V.tensor_scalar(zy[:], ascy[:], 12.0, bwd16[:, 1:2],
                                OP.subtract, OP.mult).then_inc(vd)            # 12
                v.wait_ge(vd, 12)
                V.tensor_tensor(s2[:], zx[:], zy[:], OP.max).then_inc(vd)     # 13
                v.wait_ge(ad, 5)
                V.tensor_tensor(e1[:], ex[:], qcy[:], OP.add).then_inc(vd)    # 14
                v.wait_ge(ad, 6)
                V.tensor_tensor(e2[:], ey[:], qcx[:], OP.add).then_inc(vd)    # 15
                v.wait_ge(vd, 15)
                V.tensor_tensor(dmin[:], e1[:], e2[:], OP.min).then_inc(vd)   # 16
                v.wait_ge(vd, 16)
                V.scalar_tensor_tensor(dz[:], s2[:], 0.0, dmin[:],
                                       OP.max, OP.min).then_inc(vd)           # 17
                v.wait_ge(ts, 1)
                V.tensor_reduce(
                    mA[:], pA[:].rearrange("p (b q) -> p q b", b=BPI),
                    axis=mybir.AxisListType.X, op=OP.min).then_inc(vd)        # 18
                v.wait_ge(ts, 2)
                V.tensor_reduce(
                    mB[:], pB[:].rearrange("p (b q) -> p q b", b=BPI),
                    axis=mybir.AxisListType.X, op=OP.min).then_inc(vd)        # 19
                v.wait_ge(vd, 19)
                V.scalar_tensor_tensor(tmpm[:], mA[:], 0.0, mB[:],
                                       OP.add, OP.add,
                                       accum_out=persum[:]).then_inc(vd)      # 20
                v.wait_ge(ts, 3)
                V.tensor_copy(fin[:], pT[:]).then_inc(vd)                     # 21

            @block.tensor
            def _(t):
                t.wait_ge(pd, 2)
                t.wait_ge(vd, 17)
                nc.tensor.matmul(pA[:], dz[:, 0:128], idn16[:],
                                 is_transpose=True).then_inc(ts)
                nc.tensor.matmul(pB[:], dz[:, 128:256], idn16[:],
                                 is_transpose=True).then_inc(ts)
                t.wait_ge(pd, 3)
                t.wait_ge(vd, 20)
                nc.tensor.matmul(pT[:], persum[:], ones[:]).then_inc(ts)

    nc.compile()
    return nc


# partition row p = b*16 + q, q = n*4 + c
_P = np.arange(128)
_B_IDX = _P // (NI * CH)
_N_IDX = (_P % (NI * CH)) // CH


def shard_inputs(boxes, fragments):
    """Per-core input marshalling (layout only, no arithmetic)."""
    boxes = np.ascontiguousarray(boxes, dtype=np.float32).reshape(
        N_CORES, NI, BPI, 4)
    frag = np.ascontiguousarray(fragments, dtype=np.float32).reshape(
        N_CORES, NI, CH, CW, 2)
    in_maps = []
    for k in range(N_CORES):
        f2 = frag[k].transpose(3, 0, 1, 2).reshape(2, ROWS, CW)
        frag2 = np.ascontiguousarray(
            np.broadcast_to(f2[:, None], (2, BPI, ROWS, CW)).reshape(2, 128, CW))
        boxp = np.ascontiguousarray(boxes[k, _N_IDX, _B_IDX, :])
        in_maps.append({"frag2": frag2, "boxp": boxp})
    return in_maps


_NC = None


import os


def _get_nc():
    global _NC
    if _NC is None:
        kind = os.environ.get("COV_KERNEL", "v2")
        if os.environ.get("COV_RAW_KERNEL"):
            kind = "raw"
        if kind == "raw":
            _NC = build_nc_raw()
        elif kind == "base":
            _NC = build_nc()
        else:
            _NC = build_nc_v2()
    return _NC


def run(boxes, fragments, trace=False, **spmd_kwargs):
    nc = _get_nc()
    in_maps = shard_inputs(boxes, fragments)
    res = run_bass_kernel_spmd(nc, in_maps, list(range(N_CORES)),
                               trace=trace, **spmd_kwargs)
    parts = np.stack([np.asarray(r["out"], dtype=np.float32).reshape(-1)
                      for r in res.results])
    total = parts.sum(dtype=np.float32)
    loss = np.float32(total / np.float32(FP * N_IMG))
    return loss, res


def kernel(boxes, fragments, obj_to_img):
    loss, _ = run(boxes, fragments)
    return loss

